# revision 10
# baseline (speedup 1.0000x reference)
"""NeuralMemoryLinear fused Bass/Tile kernel for Trainium2 (8 NeuronCores).

Contract: kernel(**inputs) takes the FULL (unsharded) inputs of
nn_NeuralMemoryLinear (x, hidden, projection weights, per-head gate
params) and returns the FULL output tuple (y, W_next).

Sharding: 16 independent (batch, head) problems over 8 cores —
core c handles batch b = c//4 and heads n1 = 2*(c%4), n1+1. Each core
emits its two heads' additive y-contributions plus its W_next column
slice; the host sums y over heads and concatenates W_next slices.

The on-device program (per core, identical SPMD program, per-core data):
  xT = transpose(x_b)                      (PE transpose via identity)
  v  = x_b @ wv + bv
  qT/kT = (wq/wk slices)^T-projected from xT;  gate stats lr, lwd
  cumsum(lwd) via triangular matmul + block-offset scan
  per head:  A = v - k @ Wp^T
             mask[a,m] = k·q * exp((a+1)·lwd[m]) (triu, diag fix) * lr[a]
             y = mask^T @ A + (wd_cross ⊙ q) @ Wp^T
             Wn = A^T @ (lr·wdlast ⊙ k) + wd_cross[-1]·Wp
Matmuls run as float32r (full-rate fp32 mode on the PE).
"""
import numpy as np

import concourse.bass as bass
import concourse.bacc as bacc
import concourse.tile as tile
import concourse.mybir as mybir

F32 = mybir.dt.float32
F32R = mybir.dt.float32r
AF = mybir.ActivationFunctionType
ALU = mybir.AluOpType

BATCH, L, DIM = 2, 2048, 1024
NUM_HEAD, DH = 8, 128
N_CORES = 8


def split_drain_waits(nc, max_waits=1):
    """The walrus build in this container rejects Drain instructions with
    more than one sync wait. Split each such drain into a chain of
    single-wait drains (idempotent fence semantics make this safe)."""
    f = nc.m.functions[0]
    for bb in f.blocks:
        insts = bb.instructions
        new_list = []
        changed = False
        for inst in insts:
            si = inst.sync_info
            if (
                type(inst).__name__ == "InstDrain"
                and si is not None
                and len(si.on_wait) > max_waits
            ):
                waits = list(si.on_wait)
                for w in waits[:-max_waits]:
                    d = mybir.InstDrain(name=nc.get_next_instruction_name())
                    d.engine = inst.engine
                    d.sync_info = mybir.SyncInfo(on_wait=[w], on_update=[])
                    new_list.append(d)
                inst.sync_info = mybir.SyncInfo(
                    on_wait=waits[-max_waits:], on_update=list(si.on_update)
                )
                changed = True
            new_list.append(inst)
        if changed:
            bb.instructions = new_list


def build(L=2048, DIM=1024, repeat=1, n_cores=8, drainfix=True):
    NB = DIM // 128       # d-blocks
    NA = L // 128         # a-blocks
    MS = 256              # m-strip width
    NS = L // MS          # strips
    SB = MS // 128        # m-subtiles per strip
    DT = 512              # matmul N tile width over d
    ND = DIM // DT
    H2 = 256              # two heads of 128

    nc = bacc.Bacc("TRN2", target_bir_lowering=False, debug=False,
                   num_devices=n_cores)

    xb = nc.dram_tensor("xb", [L, DIM], F32, kind="ExternalInput")
    wv = nc.dram_tensor("wv", [DIM, DIM], F32, kind="ExternalInput")
    bv = nc.dram_tensor("bv", [1, DIM], F32, kind="ExternalInput")
    wq2 = nc.dram_tensor("wq2", [DIM, H2], F32, kind="ExternalInput")
    wk2 = nc.dram_tensor("wk2", [DIM, H2], F32, kind="ExternalInput")
    bq2 = nc.dram_tensor("bq2", [128, 2], F32, kind="ExternalInput")
    bk2 = nc.dram_tensor("bk2", [128, 2], F32, kind="ExternalInput")
    wp2 = nc.dram_tensor("wp2", [DIM, H2], F32, kind="ExternalInput")
    fcw = nc.dram_tensor("fcw", [DIM, 4], F32, kind="ExternalInput")
    fcb = nc.dram_tensor("fcb", [1, 4], F32, kind="ExternalInput")
    base = nc.dram_tensor("base", [1, 4], F32, kind="ExternalInput")
    iden = nc.dram_tensor("iden", [128, 128], F32, kind="ExternalInput")
    triuI = nc.dram_tensor("triuI", [128, 128], F32, kind="ExternalInput")
    triuS = nc.dram_tensor("triuS", [128, 128], F32, kind="ExternalInput")
    iotapp = nc.dram_tensor("iotapp", [128, NA], F32, kind="ExternalInput")

    y_out = [
        nc.dram_tensor("y1", [L, DIM], F32, kind="ExternalOutput"),
        nc.dram_tensor("y2", [L, DIM], F32, kind="ExternalOutput"),
    ]
    wn = nc.dram_tensor("wn", [DIM, H2], F32, kind="ExternalOutput")

    def r(ap):
        return ap.bitcast(F32R)

    with tile.TileContext(nc) as tc:
        def body(_iv=None):
            import contextlib
            ctx = contextlib.ExitStack()
            with ctx:
                constp = ctx.enter_context(tc.tile_pool(name="const", bufs=1))
                dramp = ctx.enter_context(
                    tc.tile_pool(name="dram", bufs=1, space="DRAM"))
                statp = ctx.enter_context(tc.tile_pool(name="stat", bufs=1))
                pst = ctx.enter_context(
                    tc.tile_pool(name="pst", bufs=2, space="PSUM"))
                psb = ctx.enter_context(
                    tc.tile_pool(name="psb", bufs=2, space="PSUM"))
                pss = ctx.enter_context(
                    tc.tile_pool(name="pss", bufs=2, space="PSUM"))
                py = ctx.enter_context(
                    tc.tile_pool(name="py", bufs=2, space="PSUM"))

                # ---- constants ----
                iden_sb = constp.tile([128, 128], F32, tag="iden", name="iden")
                nc.sync.dma_start(iden_sb[:], iden[:, :])
                idenr_sb = constp.tile([128, 128], F32R, tag="idenr", name="idenr")
                nc.sync.dma_start(idenr_sb[:], iden[:, :].bitcast(F32R))
                onesr_sb = constp.tile([1, 128], F32R, tag="onesr", name="onesr")
                nc.sync.dma_start(onesr_sb[:], triuI[0:1, :].bitcast(F32R))
                triuI_sb = constp.tile([128, 128], F32, tag="triuI", name="triuI")
                nc.sync.dma_start(triuI_sb[:], triuI[:, :])
                triuS_sb = constp.tile([128, 128], F32, tag="triuS", name="triuS")
                nc.sync.dma_start(triuS_sb[:], triuS[:, :])
                iot_sb = constp.tile([128, NA], F32, tag="iot", name="iot")
                nc.sync.dma_start(iot_sb[:], iotapp[:, :])
                fcb_bc = constp.tile([128, 4], F32, tag="fcbb", name="fcbb")
                nc.sync.dma_start(
                    fcb_bc[:], fcb[0:1, :].partition_broadcast(128)[:, 0, :])
                base_bc = constp.tile([128, 4], F32, tag="baseb", name="baseb")
                nc.sync.dma_start(
                    base_bc[:], base[0:1, :].partition_broadcast(128)[:, 0, :])
                bq_sb = constp.tile([128, 2], F32, tag="bq", name="bq")
                nc.sync.dma_start(bq_sb[:], bq2[:, :])
                bk_sb = constp.tile([128, 2], F32, tag="bk", name="bk")
                nc.sync.dma_start(bk_sb[:], bk2[:, :])
                wp_sb = constp.tile([128, NB * H2], F32, tag="wp", name="wp")
                for j in range(NB):
                    nc.sync.dma_start(wp_sb[:, j * H2:(j + 1) * H2],
                                      wp2[j * 128:(j + 1) * 128, :])

                copy_ctr = [0]

                def copy_alt(dst, src, rounded=False):
                    if rounded:
                        dst = dst.bitcast(F32R)
                    if copy_ctr[0] % 2 == 0:
                        nc.vector.tensor_copy(dst, src)
                    else:
                        nc.scalar.copy(dst, src)
                    copy_ctr[0] += 1

                earlyp = ctx.enter_context(
                    tc.tile_pool(name="early", bufs=1))
                v_sb = earlyp.tile([128, NA * DIM], F32, tag="v", name="v")

                with tc.tile_pool(name="xTp", bufs=1) as xTp:
                    xT_sb = xTp.tile([128, NB * L], F32, tag="xT", name="xT")

                    wv_ctx = tc.tile_pool(name="wvp", bufs=1)
                    wvp = wv_ctx.__enter__()
                    xstg_ctx = tc.tile_pool(name="xstg", bufs=2)
                    xstg = xstg_ctx.__enter__()
                    # ---- phase 1: load x, transpose to xT ----
                    for i in range(NA):
                        for half in range(2):
                            xs = xstg.tile([128, DIM // 2], F32, tag="xs", name="xs")
                            nc.sync.dma_start(
                                xs[:], xb[i * 128:(i + 1) * 128,
                                          half * (DIM // 2):(half + 1) * (DIM // 2)])
                            for jj in range(NB // 2):
                                j = half * (NB // 2) + jj
                                pt = pst.tile([128, 128], F32, tag="ptr", name="ptr")
                                nc.tensor.transpose(
                                    pt[:], xs[:, jj * 128:(jj + 1) * 128], iden_sb[:])
                                copy_alt(
                                    xT_sb[:, j * L + i * 128: j * L + (i + 1) * 128],
                                    pt[:], rounded=True)

                    # ---- phase 2: v = x @ wv + bv ----
                    wv_sb = wvp.tile([128, NB * DIM], F32R, tag="wv", name="wv")
                    for j in range(NB):
                        nc.sync.dma_start(wv_sb[:, j * DIM:(j + 1) * DIM],
                                          wv[j * 128:(j + 1) * 128, :].bitcast(F32R))
                    bv_row = wvp.tile([1, DIM], F32R, tag="bvr", name="bvr")
                    nc.sync.dma_start(bv_row[:], bv[:, :].bitcast(F32R))
                    for i in range(NA):
                        for dt_ in range(ND):
                            ps = psb.tile([128, DT], F32, tag="mm512", name="mm512")
                            for j in range(NB):
                                nc.tensor.matmul(
                                    ps[:],
                                    r(xT_sb[:, j * L + i * 128: j * L + (i + 1) * 128]),
                                    r(wv_sb[:, j * DIM + dt_ * DT: j * DIM + (dt_ + 1) * DT]),
                                    start=(j == 0), stop=False)
                            # bv via K=1 rank-1 update (ones row from triuI)
                            nc.tensor.matmul(
                                ps[:], onesr_sb[0:1, :],
                                bv_row[0:1, dt_ * DT:(dt_ + 1) * DT],
                                start=False, stop=True)
                            nc.vector.tensor_copy(
                                v_sb[:, i * DIM + dt_ * DT: i * DIM + (dt_ + 1) * DT],
                                ps[:])

                    xstg_ctx.__exit__(None, None, None)
                    wv_ctx.__exit__(None, None, None)

                    # ---- phase 3: qT/kT both heads (+bias), stats ----
                    with tc.tile_pool(name="wqk", bufs=1) as wqk:
                        wq_sb = wqk.tile([128, NB * H2], F32R, tag="wq", name="wq")
                        wk_sb = wqk.tile([128, NB * H2], F32R, tag="wk", name="wk")
                        fcw_sb = wqk.tile([128, NB * 4], F32R, tag="fcw", name="fcw")
                        for j in range(NB):
                            nc.sync.dma_start(wq_sb[:, j * H2:(j + 1) * H2],
                                              wq2[j * 128:(j + 1) * 128, :].bitcast(F32R))
                            nc.sync.dma_start(wk_sb[:, j * H2:(j + 1) * H2],
                                              wk2[j * 128:(j + 1) * 128, :].bitcast(F32R))
                            nc.sync.dma_start(fcw_sb[:, j * 4:(j + 1) * 4],
                                              fcw[j * 128:(j + 1) * 128, :].bitcast(F32R))

                        qT_sb = earlyp.tile([128, L], F32, tag="qT", name="qT")
                        kT_sb = earlyp.tile([128, L], F32, tag="kT", name="kT")
                        d_qT2 = dramp.tile([128, L], F32, tag="dq2", name="dq2")
                        d_kT2 = dramp.tile([128, L], F32, tag="dk2", name="dk2")

                        # head 1 first, spill to DRAM; head 0 stays in SBUF
                        for head in (1, 0):
                            for (w_sb, b_sb, dstT) in (
                                (wq_sb, bq_sb, qT_sb), (wk_sb, bk_sb, kT_sb)):
                                for mt in range(L // 512):
                                    ps = psb.tile([128, 512], F32, tag="mm512", name="mm512")
                                    for j in range(NB):
                                        nc.tensor.matmul(
                                            ps[:],
                                            r(w_sb[:, j * H2 + head * 128:
                                                   j * H2 + head * 128 + 128]),
                                            r(xT_sb[:, j * L + mt * 512:
                                                    j * L + (mt + 1) * 512]),
                                            start=(j == 0), stop=(j == NB - 1))
                                    nc.vector.tensor_scalar(
                                        r(dstT[:, mt * 512:(mt + 1) * 512]),
                                        ps[:], b_sb[:, head:head + 1], None,
                                        op0=ALU.add)
                            if head == 1:
                                nc.sync.dma_start(r(d_qT2[:, :]), r(qT_sb[:]))
                                nc.sync.dma_start(r(d_kT2[:, :]), r(kT_sb[:]))

                        # s4 stats: s_lr/s_wd for both heads
                        slr = [statp.tile([128, NA], F32, tag=f"slr{h}", name=f"slr{h}")
                               for h in (0, 1)]
                        swd = [statp.tile([128, NA], F32, tag=f"swd{h}", name=f"swd{h}")
                               for h in (0, 1)]
                        for i in range(NA):
                            ps4 = pss.tile([128, 4], F32, tag="psmall", name="psmall")
                            for j in range(NB):
                                nc.tensor.matmul(
                                    ps4[:],
                                    r(xT_sb[:, j * L + i * 128: j * L + (i + 1) * 128]),
                                    fcw_sb[:, j * 4:(j + 1) * 4],
                                    start=(j == 0), stop=(j == NB - 1))
                            for h in (0, 1):
                                nc.vector.tensor_tensor(
                                    slr[h][:, i:i + 1], ps4[:, 2 * h:2 * h + 1],
                                    fcb_bc[:, 2 * h:2 * h + 1], op=ALU.add)
                                nc.vector.tensor_tensor(
                                    swd[h][:, i:i + 1], ps4[:, 2 * h + 1:2 * h + 2],
                                    fcb_bc[:, 2 * h + 1:2 * h + 2], op=ALU.add)
                # xT, wv, wq/wk freed here

                # ---- per-head gate statistics ----
                lr_pp, wdc_pp, en_pp, sk_pp = [], [], [], []
                wdcl_bc = []
                d_lwd, d_wdc = [], []
                for h in (0, 1):
                    sig = statp.tile([128, NA], F32, tag=f"sig{h}", name=f"sig{h}")
                    nc.scalar.activation(sig[:], slr[h][:], AF.Sigmoid)
                    lr = statp.tile([128, NA], F32, tag=f"lr{h}", name=f"lr{h}")
                    nc.vector.tensor_scalar(
                        lr[:], sig[:], base_bc[:, 2 * h:2 * h + 1], None,
                        op0=ALU.mult)
                    sigw = statp.tile([128, NA], F32, tag=f"sigw{h}", name=f"sigw{h}")
                    nc.scalar.activation(sigw[:], swd[h][:], AF.Sigmoid)
                    z = statp.tile([128, NA], F32, tag=f"z{h}", name=f"z{h}")
                    nc.vector.tensor_scalar(
                        z[:], sigw[:], base_bc[:, 2 * h + 1:2 * h + 2], None,
                        op0=ALU.mult)
                    lwd = statp.tile([128, NA], F32, tag=f"lwd{h}", name=f"lwd{h}")
                    nc.scalar.activation(lwd[:], z[:], AF.Ln,
                                         bias=1.0, scale=-1.0)
                    # cumsum within blocks via triangular matmul
                    pc = pss.tile([128, NA], F32, tag="psmall", name="psmall")
                    nc.tensor.matmul(pc[:], triuI_sb[:], lwd[:],
                                     start=True, stop=True)
                    cin = statp.tile([128, NA], F32, tag=f"cin{h}", name=f"cin{h}")
                    nc.vector.tensor_copy(cin[:], pc[:])
                    # block offsets: exclusive scan of block totals
                    tot0 = statp.tile([1, NA], F32, tag=f"tot{h}", name=f"tot{h}")
                    nc.sync.dma_start(tot0[:], cin[127:128, :])
                    scn = statp.tile([1, NA], F32, tag=f"scn{h}", name=f"scn{h}")
                    nc.vector.tensor_tensor_scan(
                        scn[:], tot0[:], tot0[:], 0.0,
                        op0=ALU.add, op1=ALU.bypass)
                    ex = statp.tile([1, NA], F32, tag=f"ex{h}", name=f"ex{h}")
                    nc.vector.memset(ex[:], 0.0)
                    if NA > 1:
                        nc.vector.tensor_copy(ex[0:1, 1:NA], scn[0:1, 0:NA - 1])
                    ex_d = dramp.tile([1, NA], F32, tag=f"exd{h}", name=f"exd{h}")
                    nc.sync.dma_start(ex_d[:, :], ex[:])
                    exb = statp.tile([128, NA], F32, tag=f"exb{h}", name=f"exb{h}")
                    nc.sync.dma_start(
                        exb[:], ex_d[0:1, :].partition_broadcast(128)[:, 0, :])
                    C = statp.tile([128, NA], F32, tag=f"C{h}", name=f"C{h}")
                    nc.vector.tensor_tensor(C[:], cin[:], exb[:], op=ALU.add)
                    wdc = statp.tile([128, NA], F32, tag=f"wdc{h}", name=f"wdc{h}")
                    nc.scalar.activation(wdc[:], C[:], AF.Exp)
                    en = statp.tile([128, NA], F32, tag=f"en{h}", name=f"en{h}")
                    nc.scalar.activation(en[:], lwd[:], AF.Exp, scale=-1.0)
                    # wdlast = exp((a+1)*g_last), fix last element
                    # row-major copies of lwd / wd_cross in DRAM, via one
                    # PE transpose each ([128, NA] -> [NA, 128])
                    ptw = pst.tile([128, 128], F32, tag="ptr", name="ptr")
                    nc.tensor.transpose(ptw[:NA, :], lwd[:], iden_sb[:])
                    lrow = statp.tile([NA, 128], F32, tag=f"lrow{h}", name=f"lrow{h}")
                    nc.vector.tensor_copy(lrow[:], ptw[:NA, :])
                    dl = dramp.tile([NA, 128], F32, tag=f"dlwd{h}", name=f"dlwd{h}")
                    nc.sync.dma_start(dl[:, :], lrow[:])
                    ptw2 = pst.tile([128, 128], F32, tag="ptr", name="ptr")
                    nc.tensor.transpose(ptw2[:NA, :], wdc[:], iden_sb[:])
                    wrow_sb = statp.tile([NA, 128], F32, tag=f"wrow{h}", name=f"wrow{h}")
                    nc.vector.tensor_copy(wrow_sb[:], ptw2[:NA, :])
                    dw = dramp.tile([NA, 128], F32, tag=f"dwdc{h}", name=f"dwdc{h}")
                    nc.sync.dma_start(dw[:, :], wrow_sb[:])
                    gl0 = statp.tile([1, 1], F32, tag=f"gl0{h}", name=f"gl0{h}")
                    nc.sync.dma_start(gl0[:], lwd[127:128, NA - 1:NA])
                    gl_bc = statp.tile([128, 1], F32, tag=f"glb{h}", name=f"glb{h}")
                    nc.sync.dma_start(
                        gl_bc[:],
                        dl[NA - 1:NA, 127:128].partition_broadcast(128)[:, 0, :])
                    wdl = statp.tile([128, NA], F32, tag=f"wdl{h}", name=f"wdl{h}")
                    nc.scalar.activation(wdl[:], iot_sb[:], AF.Exp,
                                         scale=gl_bc[:, 0:1])
                    # fix last element: wdl[L-1] *= exp(-g_last), via partition-0 bounce
                    w0 = statp.tile([1, 1], F32, tag=f"w0{h}", name=f"w0{h}")
                    nc.sync.dma_start(w0[:], wdl[127:128, NA - 1:NA])
                    engl = statp.tile([1, 1], F32, tag=f"engl{h}", name=f"engl{h}")
                    nc.scalar.activation(engl[:], gl0[:], AF.Exp, scale=-1.0)
                    nc.vector.tensor_tensor(w0[:], w0[:], engl[:], op=ALU.mult)
                    nc.sync.dma_start(wdl[127:128, NA - 1:NA], w0[:])
                    sk = statp.tile([128, NA], F32, tag=f"sk{h}", name=f"sk{h}")
                    nc.vector.tensor_tensor(sk[:], lr[:], wdl[:], op=ALU.mult)
                    wcl = statp.tile([128, 1], F32, tag=f"wcl{h}", name=f"wcl{h}")
                    nc.sync.dma_start(
                        wcl[:],
                        dw[NA - 1:NA, 127:128].partition_broadcast(128)[:, 0, :])
                    lr_pp.append(lr); wdc_pp.append(wdc)
                    en_pp.append(en); sk_pp.append(sk); wdcl_bc.append(wcl)
                    d_lwd.append(dl); d_wdc.append(dw)

                # ---- per-head main phases ----
                latep = ctx.enter_context(tc.tile_pool(name="late", bufs=1))
                stp = ctx.enter_context(tc.tile_pool(name="stage", bufs=2))
                for head in (0, 1):
                    if head == 1:
                        qT_sb = earlyp.tile([128, L], F32, tag="qT", name="qT")
                        kT_sb = earlyp.tile([128, L], F32, tag="kT", name="kT")
                        nc.sync.dma_start(r(qT_sb[:]), r(d_qT2[:, :]))
                        nc.sync.dma_start(r(kT_sb[:]), r(d_kT2[:, :]))

                    # phase 5: wpT, A = v - k @ Wp.T, ks
                    wpT_sb = latep.tile([128, DIM], F32, tag="wpT", name="wpT")
                    for j in range(NB):
                        pt = pst.tile([128, 128], F32, tag="ptr", name="ptr")
                        nc.tensor.transpose(
                            pt[:],
                            wp_sb[:, j * H2 + head * 128: j * H2 + head * 128 + 128],
                            iden_sb[:])
                        copy_alt(wpT_sb[:, j * 128:(j + 1) * 128], pt[:],
                                 rounded=True)
                    A_sb = latep.tile([128, NA * DIM], F32, tag="A", name="A")
                    for i in range(NA):
                        for dt_ in range(ND):
                            ps = psb.tile([128, DT], F32, tag="mm512", name="mm512")
                            nc.tensor.matmul(
                                ps[:], r(kT_sb[:, i * 128:(i + 1) * 128]),
                                r(wpT_sb[:, dt_ * DT:(dt_ + 1) * DT]),
                                start=True, stop=True)
                            nc.vector.tensor_tensor(
                                r(A_sb[:, i * DIM + dt_ * DT: i * DIM + (dt_ + 1) * DT]),
                                v_sb[:, i * DIM + dt_ * DT: i * DIM + (dt_ + 1) * DT],
                                ps[:], op=ALU.subtract)
                    ks_sb = latep.tile([128, NA * 128], F32, tag="ks", name="ks")
                    for i in range(NA):
                        pt = pst.tile([128, 128], F32, tag="ptr", name="ptr")
                        nc.tensor.transpose(
                            r(pt[:]), r(kT_sb[:, i * 128:(i + 1) * 128]), idenr_sb[:])
                        nc.vector.tensor_scalar(
                            r(ks_sb[:, i * 128:(i + 1) * 128]), pt[:],
                            sk_pp[head][:, i:i + 1], None, op0=ALU.mult)

                    # phase 6: strips of the [a, m] masked-kq matrix + y
                    strip_sb = latep.tile([128, NA * MS], F32, tag="strip", name="strip")
                    for s in range(NS):
                        jmax = (s + 1) * SB
                        g_bc = stp.tile([128, MS], F32, tag="gbc", name="gbc")
                        nc.sync.dma_start(
                            g_bc[:],
                            d_lwd[head][s * SB:(s + 1) * SB, :].rearrange(
                                "i p -> (i p)").partition_broadcast(128)[:, :])
                        w_bc = stp.tile([128, MS], F32, tag="wbc", name="wbc")
                        nc.sync.dma_start(
                            w_bc[:],
                            d_wdc[head][s * SB:(s + 1) * SB, :].rearrange(
                                "i p -> (i p)").partition_broadcast(128)[:, :])

                        for j in range(jmax):
                            sl = strip_sb[:, j * MS:(j + 1) * MS]
                            nc.scalar.activation(r(sl), g_bc[:], AF.Exp,
                                                 scale=iot_sb[:, j:j + 1])
                            if s * SB <= j < (s + 1) * SB:
                                c0 = j * 128 - s * MS
                                msk = stp.tile([128, 128], F32, tag="msk", name="msk")
                                nc.vector.tensor_scalar(
                                    msk[:], iden_sb[:],
                                    en_pp[head][:, j:j + 1], None, op0=ALU.mult)
                                nc.vector.tensor_tensor(
                                    msk[:], msk[:], triuS_sb[:], op=ALU.add)
                                nc.vector.tensor_tensor(
                                    r(strip_sb[:, j * MS + c0: j * MS + c0 + 128]),
                                    strip_sb[:, j * MS + c0: j * MS + c0 + 128],
                                    msk[:], op=ALU.mult)
                        for j in range(jmax):
                            kqp = psb.tile([128, MS], F32, tag="mm512", name="mm512")
                            nc.tensor.matmul(
                                kqp[:], r(kT_sb[:, j * 128:(j + 1) * 128]),
                                r(qT_sb[:, s * MS:(s + 1) * MS]),
                                start=True, stop=True)
                            sl = strip_sb[:, j * MS:(j + 1) * MS]
                            nc.vector.scalar_tensor_tensor(
                                r(sl), in0=kqp[:], scalar=lr_pp[head][:, j:j + 1],
                                in1=sl, op0=ALU.mult, op1=ALU.mult)
                        # scale qT strip in place by wd_cross (for y_cross)
                        nc.vector.tensor_tensor(
                            r(qT_sb[:, s * MS:(s + 1) * MS]),
                            qT_sb[:, s * MS:(s + 1) * MS],
                            w_bc[:], op=ALU.mult)
                        for t in range(SB):
                            g = s * SB + t
                            for dt_ in range(ND):
                                yp = py.tile([128, DT], F32, tag="yp", name="yp")
                                for j in range(g + 1):
                                    nc.tensor.matmul(
                                        yp[:],
                                        r(strip_sb[:, j * MS + t * 128:
                                                   j * MS + t * 128 + 128]),
                                        r(A_sb[:, j * DIM + dt_ * DT:
                                               j * DIM + (dt_ + 1) * DT]),
                                        start=(j == 0), stop=False)
                                nc.tensor.matmul(
                                    yp[:], r(qT_sb[:, g * 128:(g + 1) * 128]),
                                    r(wpT_sb[:, dt_ * DT:(dt_ + 1) * DT]),
                                    start=False, stop=True)
                                yst = stp.tile([128, DT], F32, tag="yst", name="yst", bufs=3)
                                copy_alt(yst[:], yp[:])
                                nc.sync.dma_start(
                                    y_out[head][g * 128:(g + 1) * 128,
                                                dt_ * DT:(dt_ + 1) * DT],
                                    yst[:])

                    # phase 7: W_next slice.
                    # Computed transposed (WnT[h,d] = sum_a ks[a,h] A[a,d]) so
                    # the matmuls run at N=512 (fp32r full rate), then each
                    # 128x128 block is PE-transposed back to [d,h] layout.
                    wnT = stp.tile([128, DIM], F32, tag="wnT", name="wnT", bufs=1)
                    for dt_ in range(ND):
                        pw = py.tile([128, DT], F32, tag="yp", name="yp")
                        for i in range(NA):
                            nc.tensor.matmul(
                                pw[:],
                                r(ks_sb[:, i * 128:(i + 1) * 128]),
                                r(A_sb[:, i * DIM + dt_ * DT:
                                       i * DIM + (dt_ + 1) * DT]),
                                start=(i == 0), stop=(i == NA - 1))
                        copy_alt(wnT[:, dt_ * DT:(dt_ + 1) * DT], pw[:])
                    for jd in range(NB):
                        pw = pst.tile([128, 128], F32, tag="ptr", name="ptr")
                        nc.tensor.transpose(
                            pw[:], wnT[:, jd * 128:(jd + 1) * 128], iden_sb[:])
                        wst = stp.tile([128, 128], F32, tag="wnst", name="wnst")
                        nc.vector.scalar_tensor_tensor(
                            wst[:],
                            in0=wp_sb[:, jd * H2 + head * 128:
                                      jd * H2 + head * 128 + 128],
                            scalar=wdcl_bc[head][:, 0:1], in1=pw[:],
                            op0=ALU.mult, op1=ALU.add)
                        nc.sync.dma_start(
                            wn[jd * 128:(jd + 1) * 128,
                               head * 128:(head + 1) * 128], wst[:])

        if repeat == 1:
            body()
        else:
            with tc.For_i(0, repeat, 1) as iv:
                body(iv)

    nc.compile()
    if drainfix:
        split_drain_waits(nc)
    return nc


# ---------------- host-side orchestration ----------------

def make_consts(L, NA):
    iden = np.eye(128, dtype=np.float32)
    triuI = np.triu(np.ones((128, 128), dtype=np.float32))
    triuS = np.triu(np.ones((128, 128), dtype=np.float32), 1)
    iotapp = (np.arange(128)[:, None] + 128 * np.arange(NA)[None, :] + 1
              ).astype(np.float32)
    return iden, triuI, triuS, iotapp


def make_in_map(core, inputs, L, DIM):
    NA = L // 128
    b = core // 4
    n1 = 2 * (core % 4)
    iden, triuI, triuS, iotapp = make_consts(L, NA)
    sl = slice(n1 * 128, n1 * 128 + 256)
    f32 = np.float32
    blr = np.exp(np.asarray(inputs["log_base_lr"], f32))
    bwd = np.exp(np.asarray(inputs["log_base_wd"], f32))
    a = {k: np.asarray(v, f32) for k, v in inputs.items()}
    return {
        "xb": np.ascontiguousarray(a["x"][b]),
        "wv": np.ascontiguousarray(a["wv"]),
        "bv": a["bv"].reshape(1, -1),
        "wq2": np.ascontiguousarray(a["wq"][:, sl]),
        "wk2": np.ascontiguousarray(a["wk"][:, sl]),
        "bq2": np.ascontiguousarray(a["bq"][sl].reshape(2, 128).T),
        "bk2": np.ascontiguousarray(a["bk"][sl].reshape(2, 128).T),
        "wp2": np.ascontiguousarray(a["hidden"][b][:, sl]),
        "fcw": np.ascontiguousarray(
            np.stack([a["fc_lr_w"][:, n1], a["fc_wd_w"][:, n1],
                      a["fc_lr_w"][:, n1 + 1], a["fc_wd_w"][:, n1 + 1]],
                     axis=1)),
        "fcb": np.array([[a["fc_lr_b"][n1], a["fc_wd_b"][n1],
                          a["fc_lr_b"][n1 + 1], a["fc_wd_b"][n1 + 1]]], f32),
        "base": np.array([[blr[n1], bwd[n1], blr[n1 + 1], bwd[n1 + 1]]], f32),
        "iden": iden, "triuI": triuI, "triuS": triuS, "iotapp": iotapp,
    }


class SpmdRunner:
    """Compile the Bass program once via PJRT shard_map and allow repeated
    executions (adapted from concourse.bass2jax.run_bass_via_pjrt)."""

    def __init__(self, nc, n_cores):
        import jax
        from jax.sharding import Mesh, PartitionSpec
        try:
            from jax.experimental.shard_map import shard_map
        except ImportError:
            from jax.shard_map import shard_map
        from concourse.bass2jax import (
            _bass_exec_p, install_neuronx_cc_hook, partition_id_tensor)

        install_neuronx_cc_hook()
        self.jax = jax
        self.n_cores = n_cores
        partition_name = (nc.partition_id_tensor.name
                          if nc.partition_id_tensor else None)
        in_names, out_names, out_avals, zero_outs = [], [], [], []
        for alloc in nc.m.functions[0].allocations:
            if not isinstance(alloc, mybir.MemoryLocationSet):
                continue
            name = alloc.memorylocations[0].name
            if alloc.kind == "ExternalInput":
                if name != partition_name:
                    in_names.append(name)
            elif alloc.kind == "ExternalOutput":
                out_names.append(name)
                shape = tuple(alloc.tensor_shape)
                dtype = mybir.dt.np(alloc.dtype)
                out_avals.append(jax.core.ShapedArray(shape, dtype))
                zero_outs.append(np.zeros(shape, dtype))
        self.in_names, self.out_names = in_names, out_names
        self.out_avals, self.zero_outs = out_avals, zero_outs
        n_params = len(in_names)
        self.n_params = n_params
        all_in_names = list(in_names) + list(out_names)
        if partition_name is not None:
            all_in_names.append(partition_name)

        def _body(*args):
            operands = list(args)
            if partition_name is not None:
                operands.append(partition_id_tensor())
            outs = _bass_exec_p.bind(
                *operands,
                out_avals=tuple(out_avals),
                in_names=tuple(all_in_names),
                out_names=tuple(out_names),
                lowering_input_output_aliases=(),
                sim_require_finite=False,
                sim_require_nnan=False,
                nc=nc,
            )
            return tuple(outs)

        devices = jax.devices()[:n_cores]
        assert len(devices) == n_cores, (
            f"need {n_cores} devices, found {len(jax.devices())}")
        mesh = Mesh(np.asarray(devices), ("core",))
        n_outs = len(out_avals)
        in_specs = (PartitionSpec("core"),) * (n_params + n_outs)
        out_specs = (PartitionSpec("core"),) * n_outs
        self.fn = jax.jit(
            shard_map(_body, mesh=mesh, in_specs=in_specs,
                      out_specs=out_specs, check_rep=False),
            keep_unused=True)
        self._dev_args = None

    def stage_inputs(self, in_maps):
        n = self.n_cores
        per_core = [[np.asarray(m[name]) for name in self.in_names]
                    for m in in_maps]
        concat_in = [
            np.concatenate([per_core[c][i] for c in range(n)], axis=0)
            for i in range(self.n_params)
        ]
        concat_zeros = [
            np.zeros((n * z.shape[0], *z.shape[1:]), z.dtype)
            for z in self.zero_outs
        ]
        self._dev_args = self.jax.device_put(concat_in + concat_zeros)
        self.jax.block_until_ready(self._dev_args)

    def run(self):
        out = self.fn(*self._dev_args)
        self.jax.block_until_ready(out)
        return out

    def outputs(self, out_arrs):
        n = self.n_cores
        return [
            {
                name: np.asarray(out_arrs[i]).reshape(
                    n, *self.out_avals[i].shape)[c]
                for i, name in enumerate(self.out_names)
            }
            for c in range(n)
        ]


_RUNNER_CACHE = {}


def get_runner(repeat=1):
    key = repeat
    if key not in _RUNNER_CACHE:
        nc = build(L=L, DIM=DIM, repeat=repeat, n_cores=N_CORES)
        _RUNNER_CACHE[key] = SpmdRunner(nc, N_CORES)
    return _RUNNER_CACHE[key]


def kernel(**inputs):
    """Full-problem entry point: returns (y, W_next) as float32 arrays."""
    runner = get_runner(repeat=1)
    in_maps = [make_in_map(c, inputs, L, DIM) for c in range(N_CORES)]
    runner.stage_inputs(in_maps)
    res = runner.outputs(runner.run())
    y = np.zeros((BATCH, L, DIM), np.float32)
    w_next = np.zeros((BATCH, DIM, NUM_HEAD * DH), np.float32)
    for c in range(N_CORES):
        b = c // 4
        n1 = 2 * (c % 4)
        y[b] += res[c]["y1"]
        y[b] += res[c]["y2"]
        w_next[b][:, n1 * 128:n1 * 128 + 256] = res[c]["wn"]
    return (y, w_next)
